# revision 10
# baseline (speedup 1.0000x reference)
"""Causal multi-head attention (B=2, T=2048, C=1024, H=16, D=64) on 8
Trainium2 NeuronCores.

Sharding: core c = 4*b + g handles batch b (2-way data parallel) and head
group g (4-way tensor parallel over the 16 heads, 4 heads per core).  Each
core computes its heads' QKV projection, causal attention, and a partial
output projection over its 256 feature columns; the host sums the 4
partials per batch (the "all-reduce" of the TP sharding) and adds the
projection bias.  The v-bias is folded into the host-side bias add
(proj_b + proj_w @ attn_b[2C:]) since a per-feature constant added to v
passes through softmax unchanged.

Device dataflow (single fused pipeline, one NEFF, SPMD on 8 cores):
 - Everything is feature-major ([feature, token]); weights arrive
   pre-transposed/pre-scaled (1/sqrt(D) folded into wq) in bf16.
 - Causal attention per 512-token chunk, software-pipelined one s-tile
   deep: scores S^T[s, t] = k^T.T @ q^T (two heads in concurrent 64-row
   groups), exp on the scalar engine (unnormalized), causal masking of
   diagonal tiles via gpsimd affine_select, then the AV and denominator
   matmuls of the PREVIOUS s-tile (concurrent column-group pairs).
 - The QKV projection of chunk ch+1 and the output projection of
   completed chunks are interleaved into the attention iterations as
   whole-PSUM-group quanta through a shared 2-bank pool, so the PE FIFO
   always has ready work while the scalar engine exponentiates.
 - Epilogue per (pair, chunk): reciprocal_approx_fast on the denominator
   PSUM bank + one multiply, both reading PSUM directly.
All matmul operands are bf16 (rounded on host / at PSUM eviction);
accumulation stays fp32 in PSUM.
"""

import numpy as np
import ml_dtypes

NUM_HEADS = 16
C = 1024
D = 64
N_CORES = 8
CW = 512   # t-chunk width (one fp32 PSUM bank)
ST = 128   # s-tile height (one partition block)

_PROG_CACHE = {}


def _build_program(T):
    import concourse.bacc as bacc
    import concourse.mybir as mybir
    import concourse.tile as tile

    dt = mybir.dt
    f32 = dt.float32
    bf16 = dt.bfloat16
    AF = mybir.ActivationFunctionType
    ALU = mybir.AluOpType

    NCH = T // CW   # number of t-chunks
    NT = T // ST    # number of s-tiles
    KX = C // 128   # contraction tiles for the QKV projection
    JW = 3 * 2 * 128  # per-k-tile width of the fused qkv weight slab (768)

    nc = bacc.Bacc("TRN2", target_bir_lowering=False, debug=False,
                   num_devices=N_CORES)

    xT = nc.dram_tensor("xT", [C, T], bf16, kind="ExternalInput").ap()
    wT = nc.dram_tensor("wT", [C, JW], bf16, kind="ExternalInput").ap()
    bg = nc.dram_tensor("bg", [JW], f32, kind="ExternalInput").ap()
    pwT = nc.dram_tensor("pwT", [2 * 128, C], bf16, kind="ExternalInput").ap()
    outT = nc.dram_tensor("outT", [C, T], f32, kind="ExternalOutput").ap()

    with tile.TileContext(nc) as tc:
        with (
            tc.tile_pool(name="const", bufs=1) as const,
            tc.tile_pool(name="acts", bufs=1) as acts,
            tc.tile_pool(name="ptiles", bufs=8) as ptiles,
            tc.tile_pool(name="xslab", bufs=16) as xslab,
            tc.tile_pool(name="small", bufs=2) as small,
            tc.tile_pool(name="prstage", bufs=6) as prstage,
            tc.tile_pool(name="sc_ps", bufs=2, space="PSUM") as sc_psp,
            tc.tile_pool(name="av_ps", bufs=1, space="PSUM") as av_psp,
            tc.tile_pool(name="dn_ps", bufs=1, space="PSUM") as dn_psp,
            tc.tile_pool(name="mm_ps", bufs=2, space="PSUM") as mm_psp,
        ):
            # ---- constants; weight DMAs split across both queues so the
            # first chunk's m-tile groups (which need all KX blocks) are
            # not serialized behind one queue ----
            w_sb = [const.tile([128, JW], bf16, name=f"w_sb{kc}",
                               tag=f"w_sb{kc}") for kc in range(KX)]
            for kc in range(KX):
                q = nc.scalar if kc % 2 else nc.sync
                q.dma_start(out=w_sb[kc], in_=wT[kc * 128:(kc + 1) * 128, :])
            b_sb = const.tile([128, 6], f32, name="b_sb")
            nc.scalar.dma_start(out=b_sb,
                                in_=bg.rearrange("(m p) -> p m", p=128))
            pw_sb = [const.tile([128, C], bf16, name=f"pw_sb{kt}",
                                tag=f"pw_sb{kt}") for kt in range(2)]
            for kt in range(2):
                nc.scalar.dma_start(out=pw_sb[kt],
                                    in_=pwT[kt * 128:(kt + 1) * 128, :])
            ones_f32 = const.tile([128, 128], f32, name="ones_f32")
            nc.vector.memset(ones_f32, 1.0)
            ones_bf = const.tile([128, 128], bf16, name="ones_bf")
            nc.vector.tensor_copy(out=ones_bf, in_=ones_f32)

            # persistent activations: q/k per head-pair, v (natural,
            # both pairs in one tile), av^T
            q_sb = [acts.tile([128, T], bf16, name=f"q_sb{p}", tag=f"q_sb{p}")
                    for p in range(2)]
            k_sb = [acts.tile([128, T], bf16, name=f"k_sb{p}", tag=f"k_sb{p}")
                    for p in range(2)]
            v2_sb = acts.tile([128, 2 * T], bf16, name="v2_sb", tag="v2_sb")
            av_sb = [acts.tile([128, T], bf16, name=f"av_sb{p}",
                               tag=f"av_sb{p}") for p in range(2)]

            # PE warmup: dependency-free matmuls run while the input DMAs
            # land, so the HAM clock gate is at 2.4 GHz when the first
            # projection matmul issues.
            for wi in range(56):
                wu = sc_psp.tile([128, 128], f32, name="wu", tag="sc_ps")
                nc.tensor.matmul(wu, lhsT=ones_bf, rhs=ones_bf,
                                 start=True, stop=True)

            qkv_dst = q_sb + k_sb  # m-tiles: q0 q1 k0 k1

            def slab_dmas(ch):
                slabs = []
                for kc in range(KX):
                    sl = xslab.tile([128, CW], bf16, name=f"slab{kc}",
                                    tag="slab")
                    q = nc.scalar if kc % 2 else nc.sync
                    q.dma_start(
                        out=sl, in_=xT[kc * 128:(kc + 1) * 128,
                                       ch * CW:(ch + 1) * CW])
                    slabs.append(sl)
                return slabs

            def qk_gen(ch, slabs):
                for mt in range(4):
                    ps = mm_psp.tile([128, CW], f32, name="qkv_ps",
                                     tag="mm_ps")
                    for kc in range(KX):
                        nc.tensor.matmul(
                            ps,
                            lhsT=w_sb[kc][:, mt * 128:(mt + 1) * 128],
                            rhs=slabs[kc],
                            start=(kc == 0), stop=(kc == KX - 1))
                    nc.vector.tensor_scalar_add(
                        out=qkv_dst[mt][:, ch * CW:(ch + 1) * CW],
                        in0=ps, scalar1=b_sb[:, mt:mt + 1])
                    yield

            def v_gen(ch, slabs):
                # v in natural [s, d] layout: x-slab block stationary;
                # both pairs (256 features) evicted in one copy.  No bias
                # (folded into the host-side projection bias).
                for r in range(4):
                    i = 4 * ch + r
                    v_ps = mm_psp.tile([128, 2 * 128], f32,
                                       name="v_ps", tag="mm_ps")
                    for kc in range(KX):
                        nc.tensor.matmul(
                            v_ps,
                            lhsT=slabs[kc][:, r * 128:(r + 1) * 128],
                            rhs=w_sb[kc][:, 4 * 128:6 * 128],
                            start=(kc == 0), stop=(kc == KX - 1))
                    nc.vector.tensor_copy(
                        out=v2_sb[:, i * 256:(i + 1) * 256], in_=v_ps)
                    yield

            def qkv_gen(ch):
                slabs = slab_dmas(ch)
                yield from qk_gen(ch, slabs)
                yield from v_gen(ch, slabs)

            # flat software pipeline over (chunk, pair, s-tile):
            # av/dn matmuls trail scores/exp by one iteration, across
            # chunk and pair boundaries.
            state = {}  # (p, ch) -> (av_ps, dn_ps)
            proj_queue = []
            filler_gens = []
            proj_n = [0]

            def av_dn(p_, ch_, i_, pp_sb):
                av_ps, dn_ps = state[(p_, ch_)]
                n_i_ = 4 * ch_ + 4
                mm = i_ - 4 * ch_
                c0 = 128 * mm if mm > 0 else 0
                last = (i_ == n_i_ - 1)
                for h2 in range(2):
                    nc.tensor.matmul(
                        av_ps[h2 * 64:(h2 + 1) * 64, c0:CW],
                        lhsT=v2_sb[:, i_ * 256 + p_ * 128 + h2 * 64:
                                   i_ * 256 + p_ * 128 + (h2 + 1) * 64],
                        rhs=pp_sb[:, h2 * CW + c0:(h2 + 1) * CW],
                        start=(i_ == 0), stop=last,
                        skip_group_check=True)
                for h2 in range(2):
                    nc.tensor.matmul(
                        dn_ps[h2 * 64:(h2 + 1) * 64, c0:CW],
                        lhsT=ones_bf[:, 0:64],
                        rhs=pp_sb[:, h2 * CW + c0:(h2 + 1) * CW],
                        start=(i_ == 0), stop=last,
                        skip_group_check=True)

            def epilogue(p_, ch_):
                av_ps, dn_ps = state.pop((p_, ch_))
                rc = small.tile([128, CW], f32, name="rc", tag="rc", bufs=3)
                nc.vector.reciprocal_approx_fast(out=rc, in_=dn_ps)
                nc.vector.tensor_mul(
                    av_sb[p_][:, ch_ * CW:(ch_ + 1) * CW], av_ps, rc)
                if p_ == 1:
                    proj_queue.extend((mt, ch_) for mt in range(8))

            def proj_item(mt, chp):
                ps = mm_psp.tile([128, CW], f32, name="pr_ps", tag="mm_ps")
                for kt in range(2):
                    nc.tensor.matmul(
                        ps,
                        lhsT=pw_sb[kt][:, mt * 128:(mt + 1) * 128],
                        rhs=av_sb[kt][:, chp * CW:(chp + 1) * CW],
                        start=(kt == 0), stop=(kt == 1))
                stage = prstage.tile([128, CW], f32, name="pr_stage",
                                     tag="pr_stage")
                nc.vector.tensor_copy(out=stage, in_=ps)
                q = nc.scalar if proj_n[0] % 2 else nc.sync
                proj_n[0] += 1
                q.dma_start(
                    out=outT[mt * 128:(mt + 1) * 128,
                             chp * CW:(chp + 1) * CW],
                    in_=stage)

            def emit_filler():
                while filler_gens:
                    try:
                        next(filler_gens[0])
                        return
                    except StopIteration:
                        filler_gens.pop(0)
                if proj_queue:
                    proj_item(*proj_queue.pop(0))

            def drain_gens():
                while filler_gens:
                    for _ in filler_gens.pop(0):
                        pass

            carried = None  # (p, ch, i, p_sb)
            # chunk 0: the q/k m-tiles are an unavoidable burst, but the
            # v groups defer into chunk 0's attention as filler (av trails
            # exp by one iteration, so v s-tile i lands just in time)
            slabs0 = slab_dmas(0)
            for _ in qk_gen(0, slabs0):
                pass
            filler_gens.append(v_gen(0, slabs0))
            for ch in range(NCH):
                if ch + 1 < NCH:
                    filler_gens.append(qkv_gen(ch + 1))
                for p in range(2):
                    for i in range(4 * ch + 4):
                        if i == 0:
                            av_ps = av_psp.tile([128, CW], f32,
                                                name="av_ps", tag="av")
                            dn_ps = dn_psp.tile([128, CW], f32,
                                                name="dn_ps", tag="dn")
                            state[(p, ch)] = (av_ps, dn_ps)
                        m = i - 4 * ch
                        col0 = 128 * m if m > 0 else 0
                        sc_ps = sc_psp.tile([128, 2 * CW], f32,
                                            name="sc_ps", tag="sc_ps")
                        for h2 in range(2):
                            nc.tensor.matmul(
                                sc_ps[:, h2 * CW + col0:(h2 + 1) * CW],
                                lhsT=k_sb[p][h2 * 64:(h2 + 1) * 64,
                                             i * 128:(i + 1) * 128],
                                rhs=q_sb[p][h2 * 64:(h2 + 1) * 64,
                                            ch * CW + col0:
                                            (ch + 1) * CW],
                                start=True, stop=True)
                        p_sb = ptiles.tile([128, 2 * CW], bf16,
                                           name="p_sb", tag="p_sb")
                        sc3 = sc_ps.rearrange("q (h w) -> q h w", h=2)
                        p3 = p_sb.rearrange("q (h w) -> q h w", h=2)
                        if col0 == 0:
                            # flat 1D free-dim AP: one contiguous run
                            nc.scalar.activation(
                                out=p_sb, in_=sc_ps, func=AF.Exp)
                        else:
                            nc.scalar.activation(
                                out=p3[:, :, col0:CW],
                                in_=sc3[:, :, col0:CW], func=AF.Exp)
                        if m >= 0:
                            blk = p3[:, :, col0:col0 + 128]
                            nc.gpsimd.affine_select(
                                out=blk, in_=blk,
                                pattern=[[0, 2], [1, 128]],
                                channel_multiplier=-1, base=0,
                                compare_op=ALU.is_ge, fill=0.0)
                        emit_filler()
                        if carried is not None:
                            cp, cch, ci, cpsb = carried
                            av_dn(cp, cch, ci, cpsb)
                            if ci == 4 * cch + 3:
                                epilogue(cp, cch)
                        carried = (p, ch, i, p_sb)
                # burst any un-absorbed qkv of the next chunk before its
                # attention starts
                drain_gens()
            # tail: last carried av/dn + epilogue, then the final chunk's
            # (and any remaining) projection items
            cp, cch, ci, cpsb = carried
            av_dn(cp, cch, ci, cpsb)
            epilogue(cp, cch)
            while proj_queue:
                proj_item(*proj_queue.pop(0))

    nc.compile()
    return nc


def _get_program(T):
    if T not in _PROG_CACHE:
        _PROG_CACHE[T] = _build_program(T)
    return _PROG_CACHE[T]


def _prep_inputs(x, attn_w, attn_b, proj_w):
    """Host-side sharding/layout prep. Returns per-core in_maps."""
    B, T, C_ = x.shape
    bf = ml_dtypes.bfloat16
    scale = 1.0 / np.sqrt(D)
    xTs = [np.ascontiguousarray(x[b].T.astype(bf)) for b in range(B)]
    in_maps = []
    for c in range(N_CORES):
        b, g = divmod(c, 4)
        r0 = 256 * g
        wq = attn_w[r0:r0 + 256] * scale
        wk = attn_w[C_ + r0:C_ + r0 + 256]
        wv = attn_w[2 * C_ + r0:2 * C_ + r0 + 256]
        wgT = np.ascontiguousarray(
            np.concatenate([wq, wk, wv], axis=0).T.astype(bf))
        bgv = np.concatenate([attn_b[r0:r0 + 256] * scale,
                              attn_b[C_ + r0:C_ + r0 + 256],
                              np.zeros(256, dtype=np.float32)])
        pwTg = np.ascontiguousarray(proj_w[:, r0:r0 + 256].T.astype(bf))
        in_maps.append({
            "xT": xTs[b],
            "wT": wgT,
            "bg": bgv.astype(np.float32),
            "pwT": pwTg,
        })
    return in_maps


def _gather(results, bias, B, T):
    out = np.empty((B, T, C), dtype=np.float32)
    for b in range(B):
        acc = results[4 * b]["outT"].astype(np.float32).copy()
        for g in range(1, 4):
            acc += results[4 * b + g]["outT"]
        out[b] = acc.T + bias[None, :]
    return out


def kernel(x, attn_w, attn_b, proj_w, proj_b, _trace=False):
    from concourse.bass_utils import run_bass_kernel_spmd
    x = np.asarray(x, dtype=np.float32)
    attn_w = np.asarray(attn_w, dtype=np.float32)
    attn_b = np.asarray(attn_b, dtype=np.float32)
    proj_w = np.asarray(proj_w, dtype=np.float32)
    proj_b = np.asarray(proj_b, dtype=np.float32)

    B, T, _ = x.shape
    nc = _get_program(T)
    in_maps = _prep_inputs(x, attn_w, attn_b, proj_w)
    # the v-bias passes through softmax as a per-feature constant, so it
    # folds into the host-side bias add of the output projection
    bias = proj_b + proj_w @ attn_b[2 * C:3 * C]
    res = run_bass_kernel_spmd(nc, in_maps, core_ids=list(range(N_CORES)),
                               trace=_trace)
    out = _gather(res.results, bias, B, T)
    if _trace:
        return out, res
    return out
